# revision 8
# baseline (speedup 1.0000x reference)
"""AdaptiveGridMerger Trainium2 kernel.

Math: the reference scatters x[b,c,:] into a flat 8x8 grid with bilinear
(4-corner) weights from positions[b,c,:], then matmuls grid_weights
GW [270,64]. The scatter matrix S_b [64,306] (column c = the bilinear
hat weights of channel c) is tiny and depends only on positions, so it
is built on the HOST. The tail output rows 256:270 are folded into it:
  st78[c, 0:64]  = S_b[:, c]
  st78[c, 64:78] = (S_b.T @ GW[256:270].T)[c]   (Wtail fold)
so mm1 (lhsT=st78) produces gv[0:64] = S@x AND gv[64:78] = out[256:270]
in one pass. mm2 (lhsT=GW[0:256].T) produces out[0:256] from gv[0:64].

Engine budget (the binding constraints, measured on HW):
- dma_start occupies the ISSUING engine ~0.6us + ~0.7us/MB (HWDGE
  descgen). Alternating reads across two rings creates cross-ring
  same-tile WAW semaphore stalls, so ALL reads ride the SP ring as 13
  sequential dispatches (~8us SP occupancy, descgen feeds ~700 GB/s).
- One busy ring saturates HBM (~360-440 GB/s observed); b0 writes ride
  SWDGE (GpSimd) and b1 writes ride SP (idle once read dispatch ends)
  so write descgen never contends with evacs.
- DVE+ACT are reserved for PSUM->SBUF evacs (~1.2us per [128,1024]);
  they are the output-production ceiling (~290 GB/s combined), so both
  must run evacs nearly back-to-back from first data to the end.
- Reads are column-interleaved per T-half in consumption order so the
  first output quarter completes ~1.5MB into the read stream; st/gw
  ride the first read; the 50 tail channels are packed per T-half as
  [50+50 rows, 1024] padded tiles (rows 0-49 = even quarter, 64-113 =
  odd quarter) so every read runs full-width on all 16 SDMA engines.

PE: mm1 accumulates gv quarters ([78,1024] f32, 2 PSUM banks) with
group order ch0(start) -> tail -> ch1(stop) matching read arrival;
per quarter: gvt evac -> mm2 -> out evac -> write. b1 mm1 groups
interleave into b0 mm2 evac-gaps to keep PE dense (HAM warm). 12 spin
matmuls bridge the ~3.4us HAM cold window from t0 to first data.

Sharding: data-parallel over batch, 2 batches per core.
"""

import numpy as np

import concourse.bass as bass
import concourse.bacc as bacc
import concourse.mybir as mybir
from concourse import tile
from concourse.bass_utils import run_bass_kernel_spmd

B, C, T = 16, 306, 4096
M, G, GS = 270, 64, 8
N_CORES = 8
BL = B // N_CORES  # batches per core

W78 = G + 14          # st block width: 64 grid cols + 14 folded tail cols
XC = T // 2
STB = 3 * W78         # st cols per batch (ch0, ch1, tail blocks)
SC = XC               # st base col inside the xa-h0 pack
GWC = SC + BL * STB   # gw halves base col
XA = GWC + 2 * 128    # xa pack width: 2048 + 468 + 256 = 2772
T_PS = 512
TQ = 1024
N_SPIN = 12

MM_DTYPE = mybir.dt.bfloat16
NP_MM = mybir.dt.np(MM_DTYPE)
FP32 = mybir.dt.float32


def build_nc():
    nc = bacc.Bacc()
    # xa: b0 ch0 T-half0 + st/gw pack.  x0: [b, half] ch0 rest.
    xa_ext = nc.declare_dram_parameter("xa", [128, XA], MM_DTYPE, isOutput=False)
    x0_ext = nc.declare_dram_parameter("x0", [2 * BL - 1, 128, XC], MM_DTYPE, isOutput=False)
    xt_ext = nc.declare_dram_parameter("xt", [BL, 2, 128, TQ], MM_DTYPE, isOutput=False)
    x1_ext = nc.declare_dram_parameter("x1", [BL, 128, T], MM_DTYPE, isOutput=False)
    out_ext = nc.declare_dram_parameter("out", [BL, M, T], MM_DTYPE, isOutput=True)

    with tile.TileContext(nc) as tc:
        with (
            tc.tile_pool(name="const", bufs=1) as constp,
            tc.tile_pool(name="xp", bufs=1) as xp,
            tc.tile_pool(name="gvt", bufs=2) as gvtp,
            tc.tile_pool(name="op", bufs=6) as outp,
            tc.tile_pool(name="ps", bufs=4, space=bass.MemorySpace.PSUM) as psp,
        ):
            # PE clock pre-ramp: keep PE busy from t0 until first data so
            # the HAM cold window is burned on dummy work.
            dummy = constp.tile([128, T_PS], MM_DTYPE, tag="dummy")
            nc.vector.memset(dummy[:], 0.0)
            spin_ps = psp.tile([128, TQ], FP32, tag="pb", name="spin_ps")
            for _ in range(N_SPIN):
                nc.tensor.matmul(
                    spin_ps[:, :T_PS], dummy[:, :128], dummy[:], start=True, stop=True
                )

            xa = xp.tile([128, XA], MM_DTYPE, tag="xa", name="xa")
            xc0 = {}  # (b, half) -> [128, XC] ch0 cols (b0 h0 lives in xa)
            xts = {}  # (b, half) -> [128, TQ] packed tail
            xc1 = {}  # b -> [128, T] ch1
            for b in range(BL):
                for h in range(2):
                    if (b, h) != (0, 0):
                        xc0[(b, h)] = xp.tile(
                            [128, XC], MM_DTYPE, tag=f"x0_{b}{h}", name=f"x0_{b}{h}"
                        )
                    xts[(b, h)] = xp.tile(
                        [128, TQ], MM_DTYPE, tag=f"xt{b}{h}", name=f"xt{b}{h}"
                    )
                xc1[b] = xp.tile([128, T], MM_DTYPE, tag=f"x1_{b}", name=f"x1_{b}")
            xc0[(0, 0)] = xa

            # ---- reads: all on the SP ring, column-interleaved in
            # ---- consumption order
            def reads_half(b, h):
                if (b, h) == (0, 0):
                    nc.sync.dma_start(out=xa[:], in_=xa_ext[:])
                else:
                    nc.sync.dma_start(out=xc0[(b, h)][:], in_=x0_ext[2 * b + h - 1])
                nc.sync.dma_start(out=xts[(b, h)][:], in_=xt_ext[b, h])
                if (b, h) == (1, 1):  # split the last read for tail latency
                    for qq in range(2):
                        nc.sync.dma_start(
                            out=xc1[b][:, (2 + qq) * TQ : (3 + qq) * TQ],
                            in_=x1_ext[b, :, (2 + qq) * TQ : (3 + qq) * TQ],
                        )
                else:
                    nc.sync.dma_start(
                        out=xc1[b][:, h * XC : (h + 1) * XC],
                        in_=x1_ext[b, :, h * XC : (h + 1) * XC],
                    )

            for b in range(BL):
                for h in range(2):
                    reads_half(b, h)

            k_state = {"k": 0}

            def evac(dst, src):
                if k_state["k"] % 2 == 0:
                    nc.vector.tensor_copy(dst, src)
                else:
                    nc.scalar.copy(dst, src)
                k_state["k"] += 1

            gvts = {}
            for b in range(BL):
                gvts[b] = gvtp.tile([W78, T], MM_DTYPE, tag="gvt", name=f"gvt{b}")

            gv = {}  # (b, q) -> live psum quarter accumulator

            def mm1(b, q, which, start, stop):
                # which 0: ch0 (K=128), 1: ch1 (K=128), 2: tail (K=50)
                if (b, q) not in gv:
                    gv[(b, q)] = psp.tile([128, TQ], FP32, tag="pb", name=f"gv{b}_{q}")
                for s in range(2):
                    dst = gv[(b, q)][:W78, s * T_PS : (s + 1) * T_PS]
                    if which == 2:
                        p0 = 64 * (q % 2)
                        lhs = xa[p0 : p0 + 50, SC + b * STB + 2 * W78 : SC + b * STB + 3 * W78]
                        rhs = xts[(b, q // 2)][p0 : p0 + 50, s * T_PS : (s + 1) * T_PS]
                    else:
                        lhs = xa[0:128, SC + b * STB + which * W78 : SC + b * STB + (which + 1) * W78]
                        if which == 0:
                            src = xc0[(b, q // 2)]
                            c0 = (q % 2) * TQ + s * T_PS
                        else:
                            src = xc1[b]
                            c0 = q * TQ + s * T_PS
                        rhs = src[:, c0 : c0 + T_PS]
                    nc.tensor.matmul(
                        dst, lhs, rhs, start=start, stop=stop, skip_group_check=True
                    )

            def evac_gvt(b, q):
                evac(gvts[b][:W78, q * TQ : (q + 1) * TQ], gv[(b, q)][:W78])
                del gv[(b, q)]

            def mm2_quarter(b, q):
                for mi in range(2):
                    o_ps = psp.tile([128, TQ], FP32, tag="pb", name=f"o{b}_{q}_{mi}")
                    for s in range(2):
                        c0 = q * TQ + s * T_PS
                        nc.tensor.matmul(
                            o_ps[:, s * T_PS : (s + 1) * T_PS],
                            xa[0:G, GWC + mi * 128 : GWC + (mi + 1) * 128],
                            gvts[b][0:G, c0 : c0 + T_PS],
                            start=True, stop=True, skip_group_check=True,
                        )
                    o_sb = outp.tile([128, TQ], MM_DTYPE, tag="o", name=f"ot{b}_{q}_{mi}")
                    evac(o_sb[:], o_ps[:])
                    # b0 writes ride SWDGE; b1 writes ride SP (idle once
                    # read dispatch finishes) -> parallel write descgen
                    weng = nc.gpsimd if b == 0 else nc.sync
                    weng.dma_start(
                        out=out_ext[b, mi * 128 : (mi + 1) * 128, q * TQ : (q + 1) * TQ],
                        in_=o_sb[:],
                    )

            def mm1_half(b, h):
                q0, q1 = 2 * h, 2 * h + 1
                for q in (q0, q1):
                    mm1(b, q, 0, True, False)
                for q in (q0, q1):
                    mm1(b, q, 2, False, False)
                for q in (q0, q1):
                    mm1(b, q, 1, False, True)

            def tail_write(b):
                weng = nc.gpsimd if b == 0 else nc.sync
                weng.dma_start(out=out_ext[b, 256:M, :], in_=gvts[b][G:W78, :])

            # ---- main pipeline
            mm1_half(0, 0)
            evac_gvt(0, 0)
            evac_gvt(0, 1)
            mm2_quarter(0, 0)
            mm2_quarter(0, 1)
            mm1_half(0, 1)
            evac_gvt(0, 2)
            evac_gvt(0, 3)
            mm2_quarter(0, 2)
            # fill PE while mm2(0,3)'s evacs trail
            for q in (0, 1):
                mm1(1, q, 0, True, False)
            mm2_quarter(0, 3)
            tail_write(0)
            for q in (0, 1):
                mm1(1, q, 2, False, False)
            for q in (0, 1):
                mm1(1, q, 1, False, True)
            evac_gvt(1, 0)
            evac_gvt(1, 1)
            mm2_quarter(1, 0)
            mm2_quarter(1, 1)
            mm1_half(1, 1)
            evac_gvt(1, 2)
            evac_gvt(1, 3)
            mm2_quarter(1, 2)
            mm2_quarter(1, 3)
            tail_write(1)
    nc.compile()
    return nc


def _host_st(positions, grid_weights):
    """st78 [B, C, 78] f32: bilinear hat weights + folded tail rows."""
    gp = (positions.astype(np.float32) + 1.0) * (GS / 2.0)  # [B, C, 2]
    i = np.arange(GS, dtype=np.float32)
    wy = np.maximum(0.0, 1.0 - np.abs(i[None, None, :] - gp[:, :, 0:1]))
    wx = np.maximum(0.0, 1.0 - np.abs(i[None, None, :] - gp[:, :, 1:2]))
    s = (wy[:, :, :, None] * wx[:, :, None, :]).reshape(B, C, G)
    wtail = s @ grid_weights[256:M].T.astype(np.float32)  # [B, C, 14]
    return np.concatenate([s, wtail], axis=2)


def make_in_maps(x, positions, grid_weights):
    st78 = _host_st(positions, grid_weights)
    gw = np.ascontiguousarray(grid_weights[:256].T).astype(np.float32)  # [64, 256]
    x_mm = x.astype(NP_MM)
    in_maps = []
    for i in range(N_CORES):
        g0 = i * BL
        xa_pack = np.zeros((128, XA), dtype=np.float32)
        xt_pack = np.zeros((BL, 2, 128, TQ), dtype=NP_MM)
        x0_pack = np.empty((2 * BL - 1, 128, XC), dtype=NP_MM)
        for b2 in range(BL):
            gb = g0 + b2
            c0 = SC + b2 * STB
            xa_pack[:, c0 : c0 + W78] = st78[gb, 0:128]
            xa_pack[:, c0 + W78 : c0 + 2 * W78] = st78[gb, 128:256]
            xa_pack[0:50, c0 + 2 * W78 : c0 + 3 * W78] = st78[gb, 256:C]
            xa_pack[64:114, c0 + 2 * W78 : c0 + 3 * W78] = st78[gb, 256:C]
            xa_pack[0:G, GWC + b2 * 128 : GWC + (b2 + 1) * 128] = gw[
                :, b2 * 128 : (b2 + 1) * 128
            ]
            # tail pack: [b, half]: rows 0-49 = even quarter, 64-113 = odd
            xtail = x_mm[gb, 256:C].reshape(50, 4, TQ)
            for h in range(2):
                xt_pack[b2, h, 0:50] = xtail[:, 2 * h]
                xt_pack[b2, h, 64:114] = xtail[:, 2 * h + 1]
        xa_pack[:, 0:XC] = x_mm[g0, 0:128, 0:XC]
        x0_pack[0] = x_mm[g0, 0:128, XC:T]
        for h in range(2):
            x0_pack[1 + h] = x_mm[g0 + 1, 0:128, h * XC : (h + 1) * XC]
        in_maps.append(
            {
                "xa": xa_pack.astype(NP_MM),
                "x0": x0_pack,
                "xt": xt_pack,
                "x1": np.ascontiguousarray(x_mm[g0 : g0 + BL, 128:256]),
            }
        )
    return in_maps


_NC_CACHE = None


def kernel(x, positions, grid_weights):
    global _NC_CACHE
    if _NC_CACHE is None:
        _NC_CACHE = build_nc()
    nc = _NC_CACHE
    in_maps = make_in_maps(x, positions, grid_weights)
    res = run_bass_kernel_spmd(nc, in_maps, core_ids=list(range(N_CORES)))
    out = np.concatenate([r["out"] for r in res.results], axis=0)
    return np.asarray(out, dtype=np.float32)


if __name__ == "__main__":
    xs = np.random.randn(B, C, T).astype(np.float32)
    ps = np.random.uniform(-1, 0.74, (B, C, 2)).astype(np.float32)
    gw = np.random.randn(M, G).astype(np.float32)
    out = kernel(xs, ps, gw)
    print(out.shape, out.dtype)


# revision 9
# speedup vs baseline: 1.0461x; 1.0461x over previous
"""AdaptiveGridMerger Trainium2 kernel.

Math: the reference scatters x[b,c,:] into a flat 8x8 grid with bilinear
(4-corner) weights from positions[b,c,:], then matmuls grid_weights
GW [270,64]. The scatter matrix S_b [64,306] (column c = the bilinear
hat weights of channel c) is tiny and depends only on positions, so it
is built on the HOST. The tail output rows 256:270 are folded into it:
  st78[c, 0:64]  = S_b[:, c]
  st78[c, 64:78] = (S_b.T @ GW[256:270].T)[c]   (Wtail fold)
so mm1 (lhsT=st78) produces gv[0:64] = S@x AND gv[64:78] = out[256:270]
in one pass. mm2 (lhsT=GW[0:256].T) produces out[0:256] from gv[0:64].

Engine budget (the binding constraints, measured on HW):
- dma_start occupies the ISSUING engine ~0.6us + ~0.7us/MB (HWDGE
  descgen), and small DMAs starve the ring, so reads are 8 LARGE
  (0.5-1MB) full-width transfers on the SP ring in consumption order:
  per (batch, T-half) the 128 ch0 rows and the packed 50+50 tail rows
  ride ONE [128, 3072] DMA; st/gw ride read #1.
- b0 writes ride SWDGE (GpSimd), b1 writes ride SP (idle once read
  dispatch ends) so write descgen never contends with evacs.
- DVE+ACT are reserved for PSUM->SBUF evacs (~1.2-1.5us per
  [128,1024]); they are the output-production ceiling, so both must
  run evacs nearly back-to-back from first data to the end.

PE: mm1 accumulates gv quarters ([78,1024] f32, 2 PSUM banks) with
group order ch0(start) -> tail -> ch1(stop) matching read arrival;
per quarter: gvt evac -> mm2 -> out evac -> write. b1 mm1 groups
interleave into b0 mm2 evac-gaps to keep PE dense (HAM warm). 12 spin
matmuls bridge the ~3.4us HAM cold window from t0 to first data.

Sharding: data-parallel over batch, 2 batches per core.
"""

import numpy as np

import concourse.bass as bass
import concourse.bacc as bacc
import concourse.mybir as mybir
from concourse import tile
from concourse.bass_utils import run_bass_kernel_spmd

B, C, T = 16, 306, 4096
M, G, GS = 270, 64, 8
N_CORES = 8
BL = B // N_CORES  # batches per core

W78 = G + 14          # st block width: 64 grid cols + 14 folded tail cols
XC = T // 2
STB = 3 * W78         # st cols per batch (ch0, ch1, tail blocks)
XH = XC + 1024        # xh pack width: 2048 ch0 cols + 1024 packed tail
SC = XH               # st base col inside the xa pack
GWC = SC + BL * STB   # gw halves base col
XA = GWC + 2 * 128    # xa pack width: 3072 + 468 + 256 = 3796
T_PS = 512
TQ = 1024
N_SPIN = 12

MM_DTYPE = mybir.dt.bfloat16
NP_MM = mybir.dt.np(MM_DTYPE)
FP32 = mybir.dt.float32


def build_nc():
    nc = bacc.Bacc()
    # xa: (b0,h0) ch0+tail pack + st/gw.  xh: same pack for the other 3
    # (b, half) combos.  x1: ch1 rows.
    xa_ext = nc.declare_dram_parameter("xa", [128, XA], MM_DTYPE, isOutput=False)
    xh_ext = nc.declare_dram_parameter("xh", [2 * BL - 1, 128, XH], MM_DTYPE, isOutput=False)
    x1_ext = nc.declare_dram_parameter("x1", [BL, 128, T], MM_DTYPE, isOutput=False)
    out_ext = nc.declare_dram_parameter("out", [BL, M, T], MM_DTYPE, isOutput=True)

    with tile.TileContext(nc) as tc:
        with (
            tc.tile_pool(name="const", bufs=1) as constp,
            tc.tile_pool(name="xp", bufs=1) as xp,
            tc.tile_pool(name="gvt", bufs=2) as gvtp,
            tc.tile_pool(name="op", bufs=6) as outp,
            tc.tile_pool(name="ps", bufs=4, space=bass.MemorySpace.PSUM) as psp,
        ):
            # PE clock pre-ramp: keep PE busy from t0 until first data so
            # the HAM cold window is burned on dummy work.
            dummy = constp.tile([128, T_PS], MM_DTYPE, tag="dummy")
            nc.vector.memset(dummy[:], 0.0)
            spin_ps = psp.tile([128, TQ], FP32, tag="pb", name="spin_ps")
            for _ in range(N_SPIN):
                nc.tensor.matmul(
                    spin_ps[:, :T_PS], dummy[:, :128], dummy[:], start=True, stop=True
                )

            xa = xp.tile([128, XA], MM_DTYPE, tag="xa", name="xa")
            xh = {(0, 0): xa}  # (b, half) -> [128, XH] ch0+tail pack
            xc1 = {}           # b -> [128, T] ch1
            for b in range(BL):
                for h in range(2):
                    if (b, h) != (0, 0):
                        xh[(b, h)] = xp.tile(
                            [128, XH], MM_DTYPE, tag=f"xh{b}{h}", name=f"xh{b}{h}"
                        )
                xc1[b] = xp.tile([128, T], MM_DTYPE, tag=f"x1_{b}", name=f"x1_{b}")

            # ---- reads: all on the SP ring, in consumption order
            nc.sync.dma_start(out=xa[:], in_=xa_ext[:])
            nc.sync.dma_start(out=xc1[0][:, 0:XC], in_=x1_ext[0, :, 0:XC])
            nc.sync.dma_start(out=xh[(0, 1)][:], in_=xh_ext[0])
            nc.sync.dma_start(out=xc1[0][:, XC:T], in_=x1_ext[0, :, XC:T])
            nc.sync.dma_start(out=xh[(1, 0)][:], in_=xh_ext[1])
            nc.sync.dma_start(out=xc1[1][:, 0:XC], in_=x1_ext[1, :, 0:XC])
            nc.sync.dma_start(out=xh[(1, 1)][:], in_=xh_ext[2])
            # split the last read for tail latency
            nc.sync.dma_start(out=xc1[1][:, XC : XC + TQ], in_=x1_ext[1, :, XC : XC + TQ])
            nc.sync.dma_start(out=xc1[1][:, XC + TQ : T], in_=x1_ext[1, :, XC + TQ : T])

            k_state = {"k": 0}

            def evac(dst, src):
                if k_state["k"] % 2 == 0:
                    nc.vector.tensor_copy(dst, src)
                else:
                    nc.scalar.copy(dst, src)
                k_state["k"] += 1

            gvts = {}
            for b in range(BL):
                gvts[b] = gvtp.tile([W78, T], MM_DTYPE, tag="gvt", name=f"gvt{b}")

            gv = {}  # (b, q) -> live psum quarter accumulator

            def mm1(b, q, which, start, stop):
                # which 0: ch0 (K=128), 1: ch1 (K=128), 2: tail (K=50)
                if (b, q) not in gv:
                    gv[(b, q)] = psp.tile([128, TQ], FP32, tag="pb", name=f"gv{b}_{q}")
                for s in range(2):
                    dst = gv[(b, q)][:W78, s * T_PS : (s + 1) * T_PS]
                    if which == 2:
                        p0 = 64 * (q % 2)
                        lhs = xa[p0 : p0 + 50, SC + b * STB + 2 * W78 : SC + b * STB + 3 * W78]
                        rhs = xh[(b, q // 2)][p0 : p0 + 50, XC + s * T_PS : XC + (s + 1) * T_PS]
                    else:
                        lhs = xa[0:128, SC + b * STB + which * W78 : SC + b * STB + (which + 1) * W78]
                        if which == 0:
                            src = xh[(b, q // 2)]
                            c0 = (q % 2) * TQ + s * T_PS
                        else:
                            src = xc1[b]
                            c0 = q * TQ + s * T_PS
                        rhs = src[:, c0 : c0 + T_PS]
                    nc.tensor.matmul(
                        dst, lhs, rhs, start=start, stop=stop, skip_group_check=True
                    )

            def evac_gvt(b, q):
                evac(gvts[b][:W78, q * TQ : (q + 1) * TQ], gv[(b, q)][:W78])
                del gv[(b, q)]

            def mm2_quarter(b, q):
                for mi in range(2):
                    o_ps = psp.tile([128, TQ], FP32, tag="pb", name=f"o{b}_{q}_{mi}")
                    for s in range(2):
                        c0 = q * TQ + s * T_PS
                        nc.tensor.matmul(
                            o_ps[:, s * T_PS : (s + 1) * T_PS],
                            xa[0:G, GWC + mi * 128 : GWC + (mi + 1) * 128],
                            gvts[b][0:G, c0 : c0 + T_PS],
                            start=True, stop=True, skip_group_check=True,
                        )
                    o_sb = outp.tile([128, TQ], MM_DTYPE, tag="o", name=f"ot{b}_{q}_{mi}")
                    evac(o_sb[:], o_ps[:])
                    # b0 writes ride SWDGE; b1 writes ride SP (idle once
                    # read dispatch finishes) -> parallel write descgen
                    weng = nc.gpsimd if b == 0 else nc.sync
                    weng.dma_start(
                        out=out_ext[b, mi * 128 : (mi + 1) * 128, q * TQ : (q + 1) * TQ],
                        in_=o_sb[:],
                    )

            def mm1_half(b, h):
                q0, q1 = 2 * h, 2 * h + 1
                for q in (q0, q1):
                    mm1(b, q, 0, True, False)
                for q in (q0, q1):
                    mm1(b, q, 2, False, False)
                for q in (q0, q1):
                    mm1(b, q, 1, False, True)

            def tail_write(b):
                weng = nc.gpsimd if b == 0 else nc.sync
                weng.dma_start(out=out_ext[b, 256:M, :], in_=gvts[b][G:W78, :])

            # ---- main pipeline
            mm1_half(0, 0)
            evac_gvt(0, 0)
            evac_gvt(0, 1)
            mm2_quarter(0, 0)
            mm2_quarter(0, 1)
            mm1_half(0, 1)
            evac_gvt(0, 2)
            evac_gvt(0, 3)
            tail_write(0)
            mm2_quarter(0, 2)
            # fill PE while mm2(0,3)'s evacs trail
            for q in (0, 1):
                mm1(1, q, 0, True, False)
            mm2_quarter(0, 3)
            for q in (0, 1):
                mm1(1, q, 2, False, False)
            for q in (0, 1):
                mm1(1, q, 1, False, True)
            evac_gvt(1, 0)
            evac_gvt(1, 1)
            mm2_quarter(1, 0)
            mm2_quarter(1, 1)
            mm1_half(1, 1)
            evac_gvt(1, 2)
            evac_gvt(1, 3)
            tail_write(1)
            mm2_quarter(1, 2)
            mm2_quarter(1, 3)
    nc.compile()
    return nc


def _host_st(positions, grid_weights):
    """st78 [B, C, 78] f32: bilinear hat weights + folded tail rows."""
    gp = (positions.astype(np.float32) + 1.0) * (GS / 2.0)  # [B, C, 2]
    i = np.arange(GS, dtype=np.float32)
    wy = np.maximum(0.0, 1.0 - np.abs(i[None, None, :] - gp[:, :, 0:1]))
    wx = np.maximum(0.0, 1.0 - np.abs(i[None, None, :] - gp[:, :, 1:2]))
    s = (wy[:, :, :, None] * wx[:, :, None, :]).reshape(B, C, G)
    wtail = s @ grid_weights[256:M].T.astype(np.float32)  # [B, C, 14]
    return np.concatenate([s, wtail], axis=2)


def make_in_maps(x, positions, grid_weights):
    st78 = _host_st(positions, grid_weights)
    gw = np.ascontiguousarray(grid_weights[:256].T).astype(np.float32)  # [64, 256]
    x_mm = x.astype(NP_MM)
    in_maps = []
    for i in range(N_CORES):
        g0 = i * BL
        xa_pack = np.zeros((128, XA), dtype=np.float32)
        xh_pack = np.zeros((2 * BL - 1, 128, XH), dtype=NP_MM)

        def fill_half(dst2d, gb, h):
            # dst2d [128, XH]: ch0 cols + packed tail cols
            dst2d[:, 0:XC] = x_mm[gb, 0:128, h * XC : (h + 1) * XC]
            xtail = x_mm[gb, 256:C].reshape(50, 4, TQ)
            dst2d[0:50, XC : XC + TQ] = xtail[:, 2 * h]
            dst2d[64:114, XC : XC + TQ] = xtail[:, 2 * h + 1]

        for b2 in range(BL):
            gb = g0 + b2
            c0 = SC + b2 * STB
            xa_pack[:, c0 : c0 + W78] = st78[gb, 0:128]
            xa_pack[:, c0 + W78 : c0 + 2 * W78] = st78[gb, 128:256]
            xa_pack[0:50, c0 + 2 * W78 : c0 + 3 * W78] = st78[gb, 256:C]
            xa_pack[64:114, c0 + 2 * W78 : c0 + 3 * W78] = st78[gb, 256:C]
            xa_pack[0:G, GWC + b2 * 128 : GWC + (b2 + 1) * 128] = gw[
                :, b2 * 128 : (b2 + 1) * 128
            ]
        xa_half = np.zeros((128, XH), dtype=NP_MM)
        fill_half(xa_half, g0, 0)
        xa_pack[:, 0:XH] = xa_half.astype(np.float32)
        fill_half(xh_pack[0], g0, 1)
        fill_half(xh_pack[1], g0 + 1, 0)
        fill_half(xh_pack[2], g0 + 1, 1)
        in_maps.append(
            {
                "xa": xa_pack.astype(NP_MM),
                "xh": xh_pack,
                "x1": np.ascontiguousarray(x_mm[g0 : g0 + BL, 128:256]),
            }
        )
    return in_maps


_NC_CACHE = None


def kernel(x, positions, grid_weights):
    global _NC_CACHE
    if _NC_CACHE is None:
        _NC_CACHE = build_nc()
    nc = _NC_CACHE
    in_maps = make_in_maps(x, positions, grid_weights)
    res = run_bass_kernel_spmd(nc, in_maps, core_ids=list(range(N_CORES)))
    out = np.concatenate([r["out"] for r in res.results], axis=0)
    return np.asarray(out, dtype=np.float32)


if __name__ == "__main__":
    xs = np.random.randn(B, C, T).astype(np.float32)
    ps = np.random.uniform(-1, 0.74, (B, C, 2)).astype(np.float32)
    gw = np.random.randn(M, G).astype(np.float32)
    out = kernel(xs, ps, gw)
    print(out.shape, out.dtype)


# revision 12
# speedup vs baseline: 1.1892x; 1.1368x over previous
"""AdaptiveGridMerger Trainium2 kernel.

Math: the reference scatters x[b,c,:] into a flat 8x8 grid with bilinear
(4-corner) weights from positions[b,c,:], then matmuls grid_weights
GW [270,64]. The scatter matrix S_b [64,306] (column c = the bilinear
hat weights of channel c) is tiny and depends only on positions, so it
is built on the HOST. The tail output rows 256:270 are folded into it:
  st78[c, 0:64]  = S_b[:, c]
  st78[c, 64:78] = (S_b.T @ GW[256:270].T)[c]   (Wtail fold)
so mm1 (lhsT=st78) produces gv[0:64] = S@x AND gv[64:78] = out[256:270]
in one pass. mm2 (lhsT=GW[0:256].T) produces out[0:256] from gv[0:64].

Engine budget (the binding constraints, measured on HW):
- dma_start occupies the ISSUING engine ~0.6us + ~0.7us/MB (HWDGE
  descgen), and small DMAs starve the ring, so reads are 8 LARGE
  (0.5-1MB) full-width transfers on the SP ring in consumption order:
  per (batch, T-half) the 128 ch0 rows and the packed 50+50 tail rows
  ride ONE [128, 3072] DMA; st/gw ride read #1.
- b0 writes ride SWDGE (GpSimd), b1 writes ride SP (idle once read
  dispatch ends) so write descgen never contends with evacs.
- DVE+ACT are reserved for PSUM->SBUF evacs (~1.2-1.5us per
  [128,1024]); they are the output-production ceiling, so both must
  run evacs nearly back-to-back from first data to the end.

PE: mm1 accumulates gv quarters ([78,1024] f32, 2 PSUM banks) with
group order ch0(start) -> tail -> ch1(stop) matching read arrival;
per quarter: gvt evac -> mm2 -> out evac -> write. b1 mm1 groups
interleave into b0 mm2 evac-gaps to keep PE dense (HAM warm). 12 spin
matmuls bridge the ~3.4us HAM cold window from t0 to first data.

Sharding: data-parallel over batch, 2 batches per core.
"""

import numpy as np

import concourse.bass as bass
import concourse.bacc as bacc
import concourse.mybir as mybir
from concourse import tile
from concourse.bass_utils import run_bass_kernel_spmd

B, C, T = 16, 306, 4096
M, G, GS = 270, 64, 8
N_CORES = 8
BL = B // N_CORES  # batches per core

W78 = G + 14          # st block width: 64 grid cols + 14 folded tail cols
XC = T // 2
STB = 3 * W78         # st cols per batch (ch0, ch1, tail blocks)
XH = XC + 1024        # xh pack width: 2048 ch0 cols + 1024 packed tail
SC = XH               # st base col inside the xa pack
GWC = SC + BL * STB   # gw halves base col
XA = GWC + 2 * 128    # xa pack width: 3072 + 468 + 256 = 3796
T_PS = 512
TQ = 1024
N_SPIN = 12

MM_DTYPE = mybir.dt.bfloat16
NP_MM = mybir.dt.np(MM_DTYPE)
FP32 = mybir.dt.float32


def build_nc():
    nc = bacc.Bacc()
    # xa: (b0,h0) ch0+tail pack + st/gw.  xh: same pack for the other 3
    # (b, half) combos.  x1: ch1 rows.
    xa_ext = nc.declare_dram_parameter("xa", [128, XA], MM_DTYPE, isOutput=False)
    xh_ext = nc.declare_dram_parameter("xh", [2 * BL - 1, 128, XH], MM_DTYPE, isOutput=False)
    x1_ext = nc.declare_dram_parameter("x1", [BL, 128, T], MM_DTYPE, isOutput=False)
    out_ext = nc.declare_dram_parameter("out", [BL, M, T], MM_DTYPE, isOutput=True)

    with tile.TileContext(nc) as tc:
        with (
            tc.tile_pool(name="const", bufs=1) as constp,
            tc.tile_pool(name="xp", bufs=1) as xp,
            tc.tile_pool(name="gvt", bufs=2) as gvtp,
            tc.tile_pool(name="op", bufs=6) as outp,
            tc.tile_pool(name="ps", bufs=4, space=bass.MemorySpace.PSUM) as psp,
        ):
            # PE clock pre-ramp: keep PE busy from t0 until first data so
            # the HAM cold window is burned on dummy work.
            dummy = constp.tile([128, T_PS], MM_DTYPE, tag="dummy")
            nc.vector.memset(dummy[:], 0.0)
            spin_ps = psp.tile([128, TQ], FP32, tag="pb", name="spin_ps")
            for _ in range(N_SPIN):
                nc.tensor.matmul(
                    spin_ps[:, :T_PS], dummy[:, :128], dummy[:], start=True, stop=True
                )

            xa = xp.tile([128, XA], MM_DTYPE, tag="xa", name="xa")
            xh = {(0, 0): xa}  # (b, half) -> [128, XH] ch0+tail pack
            xc1 = {}           # b -> [128, T] ch1
            for b in range(BL):
                for h in range(2):
                    if (b, h) != (0, 0):
                        xh[(b, h)] = xp.tile(
                            [128, XH], MM_DTYPE, tag=f"xh{b}{h}", name=f"xh{b}{h}"
                        )
                xc1[b] = xp.tile([128, T], MM_DTYPE, tag=f"x1_{b}", name=f"x1_{b}")

            # ---- reads: all on the SP ring, in consumption order
            nc.sync.dma_start(out=xa[:], in_=xa_ext[:])
            nc.sync.dma_start(out=xc1[0][:, 0:XC], in_=x1_ext[0, :, 0:XC])
            nc.sync.dma_start(out=xh[(0, 1)][:], in_=xh_ext[0])
            nc.sync.dma_start(out=xc1[0][:, XC:T], in_=x1_ext[0, :, XC:T])
            nc.sync.dma_start(out=xh[(1, 0)][:], in_=xh_ext[1])
            nc.sync.dma_start(out=xc1[1][:, 0:XC], in_=x1_ext[1, :, 0:XC])
            nc.sync.dma_start(out=xh[(1, 1)][:], in_=xh_ext[2])
            # split the last read for tail latency
            nc.sync.dma_start(out=xc1[1][:, XC : XC + TQ], in_=x1_ext[1, :, XC : XC + TQ])
            nc.sync.dma_start(out=xc1[1][:, XC + TQ : T], in_=x1_ext[1, :, XC + TQ : T])

            k_state = {"k": 0}

            def evac(dst, src):
                if k_state["k"] % 2 == 0:
                    nc.vector.tensor_copy(dst, src)
                else:
                    nc.scalar.copy(dst, src)
                k_state["k"] += 1

            gvts = {}
            for b in range(BL):
                gvts[b] = gvtp.tile([W78, T], MM_DTYPE, tag="gvt", name=f"gvt{b}")

            gv = {}  # (b, q) -> live psum quarter accumulator

            def mm1(b, q, which, start, stop):
                # which 0: ch0 (K=128), 1: ch1 (K=128), 2: tail (K=50)
                if (b, q) not in gv:
                    gv[(b, q)] = psp.tile([128, TQ], FP32, tag="pb", name=f"gv{b}_{q}")
                for s in range(2):
                    dst = gv[(b, q)][:W78, s * T_PS : (s + 1) * T_PS]
                    if which == 2:
                        p0 = 64 * (q % 2)
                        lhs = xa[p0 : p0 + 50, SC + b * STB + 2 * W78 : SC + b * STB + 3 * W78]
                        rhs = xh[(b, q // 2)][p0 : p0 + 50, XC + s * T_PS : XC + (s + 1) * T_PS]
                    else:
                        lhs = xa[0:128, SC + b * STB + which * W78 : SC + b * STB + (which + 1) * W78]
                        if which == 0:
                            src = xh[(b, q // 2)]
                            c0 = (q % 2) * TQ + s * T_PS
                        else:
                            src = xc1[b]
                            c0 = q * TQ + s * T_PS
                        rhs = src[:, c0 : c0 + T_PS]
                    nc.tensor.matmul(
                        dst, lhs, rhs, start=start, stop=stop, skip_group_check=True
                    )

            def evac_gvt(b, q):
                evac(gvts[b][:W78, q * TQ : (q + 1) * TQ], gv[(b, q)][:W78])
                del gv[(b, q)]

            ops = {}  # (b, q, mi) -> live mm2 psum tile

            def mm2_mms(b, q):
                for mi in range(2):
                    o_ps = psp.tile([128, TQ], FP32, tag="pb", name=f"o{b}_{q}_{mi}")
                    ops[(b, q, mi)] = o_ps
                    for s in range(2):
                        c0 = q * TQ + s * T_PS
                        nc.tensor.matmul(
                            o_ps[:, s * T_PS : (s + 1) * T_PS],
                            xa[0:G, GWC + mi * 128 : GWC + (mi + 1) * 128],
                            gvts[b][0:G, c0 : c0 + T_PS],
                            start=True, stop=True, skip_group_check=True,
                        )

            def out_flush(b, q):
                for mi in range(2):
                    o_sb = outp.tile([128, TQ], MM_DTYPE, tag="o", name=f"ot{b}_{q}_{mi}")
                    evac(o_sb[:], ops.pop((b, q, mi))[:])
                    # b0 writes ride SWDGE; b1 writes ride SP (idle once
                    # read dispatch finishes) -> parallel write descgen
                    weng = nc.gpsimd if b == 0 else nc.sync
                    weng.dma_start(
                        out=out_ext[b, mi * 128 : (mi + 1) * 128, q * TQ : (q + 1) * TQ],
                        in_=o_sb[:],
                    )

            def mm2_quarter(b, q):
                mm2_mms(b, q)
                out_flush(b, q)

            def mm1_half(b, h):
                q0, q1 = 2 * h, 2 * h + 1
                for q in (q0, q1):
                    mm1(b, q, 0, True, False)
                for q in (q0, q1):
                    mm1(b, q, 2, False, False)
                for q in (q0, q1):
                    mm1(b, q, 1, False, True)

            def tail_write(b):
                weng = nc.gpsimd if b == 0 else nc.sync
                weng.dma_start(out=out_ext[b, 256:M, :], in_=gvts[b][G:W78, :])

            # ---- main pipeline.  gvt evacs (critical path: they gate mm2)
            # are ordered AHEAD of deferrable out evacs at the batch
            # transition; b1 mm1 groups fill every PE gap so HAM stays warm.
            mm1_half(0, 0)
            evac_gvt(0, 0)
            evac_gvt(0, 1)
            mm2_quarter(0, 0)
            mm2_quarter(0, 1)
            mm1_half(0, 1)
            evac_gvt(0, 2)
            evac_gvt(0, 3)
            tail_write(0)
            mm2_quarter(0, 2)
            for q in (0, 1):
                mm1(1, q, 0, True, False)
            mm2_mms(0, 3)
            for q in (0, 1):
                mm1(1, q, 2, False, False)
            for q in (0, 1):
                mm1(1, q, 1, False, True)
            evac_gvt(1, 0)
            evac_gvt(1, 1)
            out_flush(0, 3)
            for q in (2, 3):
                mm1(1, q, 0, True, False)
            for q in (2, 3):
                mm1(1, q, 2, False, False)
            mm2_mms(1, 0)
            out_flush(1, 0)
            for q in (2, 3):
                mm1(1, q, 1, False, True)
            mm2_mms(1, 1)
            evac_gvt(1, 2)
            evac_gvt(1, 3)
            out_flush(1, 1)
            tail_write(1)
            mm2_quarter(1, 2)
            mm2_quarter(1, 3)
    nc.compile()
    return nc


def _host_st(positions, grid_weights):
    """st78 [B, C, 78] f32: bilinear hat weights + folded tail rows."""
    gp = (positions.astype(np.float32) + 1.0) * (GS / 2.0)  # [B, C, 2]
    i = np.arange(GS, dtype=np.float32)
    wy = np.maximum(0.0, 1.0 - np.abs(i[None, None, :] - gp[:, :, 0:1]))
    wx = np.maximum(0.0, 1.0 - np.abs(i[None, None, :] - gp[:, :, 1:2]))
    s = (wy[:, :, :, None] * wx[:, :, None, :]).reshape(B, C, G)
    wtail = s @ grid_weights[256:M].T.astype(np.float32)  # [B, C, 14]
    return np.concatenate([s, wtail], axis=2)


def make_in_maps(x, positions, grid_weights):
    st78 = _host_st(positions, grid_weights)
    gw = np.ascontiguousarray(grid_weights[:256].T).astype(np.float32)  # [64, 256]
    x_mm = x.astype(NP_MM)
    in_maps = []
    for i in range(N_CORES):
        g0 = i * BL
        xa_pack = np.zeros((128, XA), dtype=np.float32)
        xh_pack = np.zeros((2 * BL - 1, 128, XH), dtype=NP_MM)

        def fill_half(dst2d, gb, h):
            # dst2d [128, XH]: ch0 cols + packed tail cols
            dst2d[:, 0:XC] = x_mm[gb, 0:128, h * XC : (h + 1) * XC]
            xtail = x_mm[gb, 256:C].reshape(50, 4, TQ)
            dst2d[0:50, XC : XC + TQ] = xtail[:, 2 * h]
            dst2d[64:114, XC : XC + TQ] = xtail[:, 2 * h + 1]

        for b2 in range(BL):
            gb = g0 + b2
            c0 = SC + b2 * STB
            xa_pack[:, c0 : c0 + W78] = st78[gb, 0:128]
            xa_pack[:, c0 + W78 : c0 + 2 * W78] = st78[gb, 128:256]
            xa_pack[0:50, c0 + 2 * W78 : c0 + 3 * W78] = st78[gb, 256:C]
            xa_pack[64:114, c0 + 2 * W78 : c0 + 3 * W78] = st78[gb, 256:C]
            xa_pack[0:G, GWC + b2 * 128 : GWC + (b2 + 1) * 128] = gw[
                :, b2 * 128 : (b2 + 1) * 128
            ]
        xa_half = np.zeros((128, XH), dtype=NP_MM)
        fill_half(xa_half, g0, 0)
        xa_pack[:, 0:XH] = xa_half.astype(np.float32)
        fill_half(xh_pack[0], g0, 1)
        fill_half(xh_pack[1], g0 + 1, 0)
        fill_half(xh_pack[2], g0 + 1, 1)
        in_maps.append(
            {
                "xa": xa_pack.astype(NP_MM),
                "xh": xh_pack,
                "x1": np.ascontiguousarray(x_mm[g0 : g0 + BL, 128:256]),
            }
        )
    return in_maps


_NC_CACHE = None


def kernel(x, positions, grid_weights):
    global _NC_CACHE
    if _NC_CACHE is None:
        _NC_CACHE = build_nc()
    nc = _NC_CACHE
    in_maps = make_in_maps(x, positions, grid_weights)
    res = run_bass_kernel_spmd(nc, in_maps, core_ids=list(range(N_CORES)))
    out = np.concatenate([r["out"] for r in res.results], axis=0)
    return np.asarray(out, dtype=np.float32)


if __name__ == "__main__":
    xs = np.random.randn(B, C, T).astype(np.float32)
    ps = np.random.uniform(-1, 0.74, (B, C, 2)).astype(np.float32)
    gw = np.random.randn(M, G).astype(np.float32)
    out = kernel(xs, ps, gw)
    print(out.shape, out.dtype)
